# revision 14
# baseline (speedup 1.0000x reference)
"""v3: 16x32 bin factorization with 4x-mode DVE one-hots.

Binning (fp32 ALU): q = rne(2x+4.5) via +-2^23 magic (valid coords -> 1..8);
u = rne(0.5*x1+1.5) in {1,2} is the high bit of the q1 digit, shifted +1 to
keep magic sums positive (negative sums fall in the fp32 ulp-0.5 zone and
round to half-integers). Scaled digits come straight from scaled magics --
q0x4 = rne4(8x+18) == 4*q0, q1x16 = rne16(32x+72) == 16*q1, u64 =
rne64(32x+96) == 64*u (power-of-2 scaling commutes with round-half-even) --
so the digit combines are plain tensor_tensor add/sub on Pool
(scalar_tensor_tensor is not legal on the Pool engine):

  hi = q0x4 + u          -> 16 targets {4a+1+b : a in 1..8, b in 0..1}
  lo = q1x16 + q2 - u64  -> 32 targets {16k+c-64 : k in 1..4, c in 1..8}

Out-of-range coords miss both target sets (u notin {1,2} breaks hi's mod-4
pattern; q2/k out of range miss lo's mod-16 gaps), so invalid points drop
out of the one-hot product, matching torch.histogramdd's drop semantics.

One-hot rows are tensor_scalar(is_equal) against immediate targets: all-bf16
packed SBUF operands hit the DVE 4x_2p perf mode (0.26 ns/elem). Stage-1
pairing: stationary=lo-oh (32 wide) x moving=hi-oh (16 wide), 64
accumulating matmuls per batch into PSUM regions packing 16 batches/tile.
Stage-2 (reindexed hi/lo-split bf16 W contracted over (lo,hi)) runs
per-window into disjoint ps2 column ranges so the tail stays short; the
first half-window's prep is emitted in quarter chunks to shorten the
startup ramp.
"""

import numpy as np

B, N, VR, CLS = 1024, 8192, 8, 40
NCORES = 8
BPC = B // NCORES      # 128 batches per core
PJ = N // 128          # 64 point-slots per partition per batch
HW = 8                 # batches per half-window
WIN = 16               # batches per window
NHW = BPC // HW        # 16 half-windows
NW = BPC // WIN        # 8 windows
HWS = HW * PJ          # 512 slots per half-window
WS = WIN * PJ          # 1024 slots per window
M1 = 8388608.0         # 2^23
M4 = 33554432.0        # 2^25
M16 = 134217728.0      # 2^27
M64 = 536870912.0      # 2^29

HI_T = [4.0 * a + 1.0 + uu for a in range(1, 9) for uu in (0, 1)]       # 16
LO_T = [16.0 * k + c - 64.0 for k in range(1, 5) for c in range(1, 9)]  # 32

_CACHE = {}


def _build():
    import concourse.bacc as bacc
    import concourse.mybir as mybir
    import concourse.tile as tile

    dt = mybir.dt
    op = mybir.AluOpType
    AF = mybir.ActivationFunctionType
    nc = bacc.Bacc("TRN2", target_bir_lowering=False, debug=False,
                   num_devices=NCORES)

    x_d = nc.dram_tensor("x", (128, NHW, HWS * 3), dt.float32,
                         kind="ExternalInput")
    w2_d = nc.dram_tensor("w2", (32, 2, 16 * CLS), dt.bfloat16,
                          kind="ExternalInput")
    bias_d = nc.dram_tensor("bias", (CLS, 1), dt.float32,
                            kind="ExternalInput")
    y_d = nc.dram_tensor("y", (CLS, BPC), dt.float32, kind="ExternalOutput")

    with tile.TileContext(nc) as tc:
        with (
            tc.tile_pool(name="const", bufs=1) as cpool,
            tc.tile_pool(name="xg", bufs=2) as xpool,
            tc.tile_pool(name="tt", bufs=2) as tpool,
            tc.tile_pool(name="dig", bufs=1) as dpool,
            tc.tile_pool(name="hl", bufs=2) as hlpool,
            tc.tile_pool(name="ohh", bufs=2) as ohhpool,
            tc.tile_pool(name="ohl", bufs=2) as ohlpool,
            tc.tile_pool(name="cnt", bufs=1) as cntpool,
            tc.tile_pool(name="ps1", bufs=2, space="PSUM") as ps1pool,
            tc.tile_pool(name="ps2", bufs=1, space="PSUM") as ps2pool,
        ):
            w2 = cpool.tile([32, 2, 16 * CLS], dt.bfloat16)
            nc.sync.dma_start(w2[:], w2_d[:])
            bias = cpool.tile([CLS, 1], dt.float32)
            nc.sync.dma_start(bias[:], bias_d[:])

            cnt = cntpool.tile([32, BPC, 16], dt.bfloat16)

            def prep_chunk(hwi, hi, lo_dst, c0, c1):
                """Emit DMA + coord prep + digit combines for slot range
                [c0, c1) of half-window hwi; writes hi[:, c0:c1] and
                lo_dst[:, c0:c1]."""
                cs = c1 - c0
                xg = xpool.tile([128, cs, 3], dt.float32, tag="xg")
                nc.sync.dma_start(xg[:], x_d[:, hwi, c0 * 3:c1 * 3].rearrange(
                    "p (s c) -> p s c", c=3))

                t0 = tpool.tile([128, cs], dt.float32, tag="t0")
                nc.scalar.activation(t0[:], xg[:, :, 0], AF.Copy,
                                     bias=18.0, scale=8.0)
                t1 = tpool.tile([128, cs], dt.float32, tag="t1")
                nc.scalar.activation(t1[:], xg[:, :, 1], AF.Copy,
                                     bias=72.0, scale=32.0)
                t2 = tpool.tile([128, cs], dt.float32, tag="t2")
                nc.scalar.activation(t2[:], xg[:, :, 2], AF.Copy,
                                     bias=4.5, scale=2.0)
                tu = tpool.tile([128, cs], dt.float32, tag="tu")
                nc.scalar.activation(tu[:], xg[:, :, 1], AF.Copy,
                                     bias=1.5, scale=0.5)
                tv = tpool.tile([128, cs], dt.float32, tag="t0")
                nc.scalar.activation(tv[:], xg[:, :, 1], AF.Copy,
                                     bias=96.0, scale=32.0)

                q0x4 = dpool.tile([128, cs], dt.bfloat16, tag="q0x4")
                nc.gpsimd.tensor_scalar(q0x4[:], t0[:], M4, -M4,
                                        op.add, op.add)
                q1x16 = dpool.tile([128, cs], dt.bfloat16, tag="q1x16")
                nc.gpsimd.tensor_scalar(q1x16[:], t1[:], M16, -M16,
                                        op.add, op.add)
                q2 = dpool.tile([128, cs], dt.bfloat16, tag="q2")
                nc.gpsimd.tensor_scalar(q2[:], t2[:], M1, -M1,
                                        op.add, op.add)
                u = dpool.tile([128, cs], dt.bfloat16, tag="u")
                nc.gpsimd.tensor_scalar(u[:], tu[:], M1, -M1,
                                        op.add, op.add)
                u64 = dpool.tile([128, cs], dt.bfloat16, tag="u64")
                nc.gpsimd.tensor_scalar(u64[:], tv[:], M64, -M64,
                                        op.add, op.add)

                nc.gpsimd.tensor_tensor(hi[:, c0:c1], q0x4[:], u[:], op.add)
                s1 = dpool.tile([128, cs], dt.bfloat16, tag="s1")
                nc.gpsimd.tensor_tensor(s1[:], q1x16[:], q2[:], op.add)
                nc.gpsimd.tensor_tensor(lo_dst[:, c0:c1], s1[:], u64[:],
                                        op.subtract)

            for w in range(NW):
                lo = hlpool.tile([128, WS], dt.bfloat16, tag="lo")
                ohh_tiles = []
                for h in range(2):
                    hwi = w * 2 + h
                    hi = hlpool.tile([128, HWS], dt.bfloat16, tag="hi")
                    lo_dst = lo[:, h * HWS:(h + 1) * HWS]
                    ohh = ohhpool.tile([128, 16, HWS], dt.bfloat16, tag="ohh")
                    if hwi == 0:
                        bounds = [0, 256, HWS]
                    else:
                        bounds = [0, HWS]
                    for c0, c1 in zip(bounds[:-1], bounds[1:]):
                        prep_chunk(hwi, hi, lo_dst, c0, c1)
                        nc.gpsimd.tensor_scalar(ohh[:, 0, c0:c1],
                                                hi[:, c0:c1], HI_T[0],
                                                None, op.is_equal)
                        for r in range(1, 16):
                            nc.vector.tensor_scalar(ohh[:, r, c0:c1],
                                                    hi[:, c0:c1], HI_T[r],
                                                    None, op.is_equal)
                    ohh_tiles.append(ohh)

                ohl = ohlpool.tile([128, 32, WS], dt.bfloat16, tag="ohl")
                # First/last window: emit lo rows per half-window span so PE
                # can start (resp. finish) half a window earlier.
                lob = [0, HWS, WS] if w in (0, NW - 1) else [0, WS]
                for c0, c1 in zip(lob[:-1], lob[1:]):
                    nc.gpsimd.tensor_scalar(ohl[:, 0, c0:c1],
                                            lo[:, c0:c1], LO_T[0],
                                            None, op.is_equal)
                    for r in range(1, 32):
                        nc.vector.tensor_scalar(ohl[:, r, c0:c1],
                                                lo[:, c0:c1], LO_T[r],
                                                None, op.is_equal)

                ps = ps1pool.tile([32, WIN * 16], dt.float32, tag="ps")
                for b in range(WIN):
                    ohh = ohh_tiles[b // HW]
                    sh = (b % HW) * PJ
                    sw = b * PJ
                    for j in range(PJ):
                        nc.tensor.matmul(ps[:, b * 16:(b + 1) * 16],
                                         ohl[:, :, sw + j],
                                         ohh[:, :, sh + j],
                                         start=(j == 0), stop=(j == PJ - 1))
                nc.scalar.copy(cnt[:, w * WIN:(w + 1) * WIN, :],
                               ps[:].rearrange("p (b h) -> p b h", h=16))

                if w == 0:
                    ps2 = ps2pool.tile([CLS, BPC], dt.float32)
                for half in range(2):
                    for r in range(16):
                        nc.tensor.matmul(
                            ps2[:, w * WIN:(w + 1) * WIN],
                            w2[:, half, r * CLS:(r + 1) * CLS],
                            cnt[:, w * WIN:(w + 1) * WIN, r],
                            start=(half == 0 and r == 0),
                            stop=(half == 1 and r == 15))

            out = cpool.tile([CLS, BPC], dt.float32)
            nc.vector.tensor_scalar(out[:], ps2[:], 1.0 / N, bias[:],
                                    op.mult, op.add)
            nc.sync.dma_start(y_d[:], out[:])

    nc.compile()
    return nc


def _aux_inputs(W, b):
    from ml_dtypes import bfloat16 as bf16
    # w2[lo_idx, half, r*CLS + c] = hi/lo bf16 split of W[c, lin(lo_idx, r)]
    lin = np.zeros((32, 16), np.int64)
    for r in range(16):
        a, uu = r // 2 + 1, r % 2
        for r2 in range(32):
            k, c2 = r2 // 8 + 1, r2 % 8 + 1
            lin[r2, r] = 64 * (a - 1) + 8 * (4 * uu + k - 1) + (c2 - 1)
    W2 = W[:, lin].astype(np.float32)                  # (CLS, 32, 16)
    W2 = np.ascontiguousarray(W2.transpose(1, 2, 0))   # (32, 16, CLS)
    W2h = W2.astype(bf16)
    W2l = (W2 - W2h.astype(np.float32)).astype(bf16)
    w2 = np.ascontiguousarray(
        np.stack([W2h, W2l], axis=1).reshape(32, 2, 16 * CLS))
    bias = np.asarray(b, dtype=np.float32).reshape(CLS, 1)
    return w2, bias


def kernel(x, W, b):
    from concourse.bass_utils import run_bass_kernel_spmd

    x = np.asarray(x, dtype=np.float32)
    W = np.asarray(W, dtype=np.float32)
    b = np.asarray(b, dtype=np.float32)

    if "nc" not in _CACHE:
        _CACHE["nc"] = _build()
    nc = _CACHE["nc"]

    w2, bias = _aux_inputs(W, b)
    # x (B, N, 3) -> per core [128p, NHW, HW, PJ, 3] -> (128, NHW, HWS*3)
    xs = x.reshape(NCORES, NHW, HW, 128, PJ, 3).transpose(0, 3, 1, 2, 4, 5)
    xs = np.ascontiguousarray(xs).reshape(NCORES, 128, NHW, HWS * 3)
    in_maps = [
        {"x": xs[i], "w2": w2, "bias": bias}
        for i in range(NCORES)
    ]
    res = run_bass_kernel_spmd(nc, in_maps, list(range(NCORES)))
    return np.concatenate(
        [np.asarray(res.results[i]["y"]).T for i in range(NCORES)],
        axis=0).astype(np.float32)


# revision 15
# speedup vs baseline: 1.0725x; 1.0725x over previous
"""v3: 16x32 bin factorization with 4x-mode DVE one-hots.

Binning (fp32 ALU): q = rne(2x+4.5) via +-2^23 magic (valid coords -> 1..8);
u = rne(0.5*x1+1.5) in {1,2} is the high bit of the q1 digit, shifted +1 to
keep magic sums positive (negative sums fall in the fp32 ulp-0.5 zone and
round to half-integers). Scaled digits come straight from scaled magics --
q0x4 = rne4(8x+18) == 4*q0, q1x16 = rne16(32x+72) == 16*q1, u64 =
rne64(32x+96) == 64*u (power-of-2 scaling commutes with round-half-even) --
so the digit combines are plain tensor_tensor add/sub on Pool
(scalar_tensor_tensor is not legal on the Pool engine):

  hi = q0x4 + u          -> 16 targets {4a+1+b : a in 1..8, b in 0..1}
  lo = q1x16 + q2 - u64  -> 32 targets {16k+c-64 : k in 1..4, c in 1..8}

Out-of-range coords miss both target sets (u notin {1,2} breaks hi's mod-4
pattern; q2/k out of range miss lo's mod-16 gaps), so invalid points drop
out of the one-hot product, matching torch.histogramdd's drop semantics.

One-hot rows are tensor_scalar(is_equal) against immediate targets: all-bf16
packed SBUF operands hit the DVE 4x_2p perf mode (0.26 ns/elem). Stage-1
pairing: stationary=lo-oh (32 wide) x moving=hi-oh (16 wide), 64
accumulating matmuls per batch into PSUM regions packing 16 batches/tile.
Stage-2 (reindexed hi/lo-split bf16 W contracted over (lo,hi)) runs
per-window into disjoint ps2 column ranges so the tail stays short; the
first half-window's prep is emitted in quarter chunks to shorten the
startup ramp.
"""

import numpy as np

B, N, VR, CLS = 1024, 8192, 8, 40
NCORES = 8
BPC = B // NCORES      # 128 batches per core
PJ = N // 128          # 64 point-slots per partition per batch
HW = 8                 # batches per half-window
WIN = 16               # batches per window
NHW = BPC // HW        # 16 half-windows
NW = BPC // WIN        # 8 windows
HWS = HW * PJ          # 512 slots per half-window
WS = WIN * PJ          # 1024 slots per window
M1 = 8388608.0         # 2^23
M4 = 33554432.0        # 2^25
M16 = 134217728.0      # 2^27
M64 = 536870912.0      # 2^29

HI_T = [4.0 * a + 1.0 + uu for a in range(1, 9) for uu in (0, 1)]       # 16
LO_T = [16.0 * k + c - 64.0 for k in range(1, 5) for c in range(1, 9)]  # 32

_CACHE = {}


def _build():
    import concourse.bacc as bacc
    import concourse.mybir as mybir
    import concourse.tile as tile

    dt = mybir.dt
    op = mybir.AluOpType
    AF = mybir.ActivationFunctionType
    nc = bacc.Bacc("TRN2", target_bir_lowering=False, debug=False,
                   num_devices=NCORES)

    x_d = nc.dram_tensor("x", (128, NHW, HWS * 3), dt.float32,
                         kind="ExternalInput")
    w2_d = nc.dram_tensor("w2", (32, 2, 16 * CLS), dt.bfloat16,
                          kind="ExternalInput")
    bias_d = nc.dram_tensor("bias", (CLS, 1), dt.float32,
                            kind="ExternalInput")
    y_d = nc.dram_tensor("y", (CLS, BPC), dt.float32, kind="ExternalOutput")

    with tile.TileContext(nc) as tc:
        with (
            tc.tile_pool(name="const", bufs=1) as cpool,
            tc.tile_pool(name="xg", bufs=2) as xpool,
            tc.tile_pool(name="tt", bufs=2) as tpool,
            tc.tile_pool(name="dig", bufs=1) as dpool,
            tc.tile_pool(name="hl", bufs=2) as hlpool,
            tc.tile_pool(name="ohh", bufs=2) as ohhpool,
            tc.tile_pool(name="ohl", bufs=2) as ohlpool,
            tc.tile_pool(name="cnt", bufs=1) as cntpool,
            tc.tile_pool(name="ps1", bufs=2, space="PSUM") as ps1pool,
            tc.tile_pool(name="ps2", bufs=1, space="PSUM") as ps2pool,
        ):
            w2 = cpool.tile([32, 2, 16 * CLS], dt.bfloat16)
            nc.sync.dma_start(w2[:], w2_d[:])
            bias = cpool.tile([CLS, 1], dt.float32)
            nc.sync.dma_start(bias[:], bias_d[:])

            cnt = cntpool.tile([32, BPC, 16], dt.bfloat16)

            def prep_chunk(hwi, hi, lo_dst, c0, c1):
                """Emit DMA + coord prep + digit combines for slot range
                [c0, c1) of half-window hwi; writes hi[:, c0:c1] and
                lo_dst[:, c0:c1]."""
                cs = c1 - c0
                xg = xpool.tile([128, cs, 3], dt.float32, tag="xg")
                nc.sync.dma_start(xg[:], x_d[:, hwi, c0 * 3:c1 * 3].rearrange(
                    "p (s c) -> p s c", c=3))

                t0 = tpool.tile([128, cs], dt.float32, tag="t0")
                nc.scalar.activation(t0[:], xg[:, :, 0], AF.Copy,
                                     bias=18.0, scale=8.0)
                t1 = tpool.tile([128, cs], dt.float32, tag="t1")
                nc.scalar.activation(t1[:], xg[:, :, 1], AF.Copy,
                                     bias=72.0, scale=32.0)
                t2 = tpool.tile([128, cs], dt.float32, tag="t2")
                nc.scalar.activation(t2[:], xg[:, :, 2], AF.Copy,
                                     bias=4.5, scale=2.0)
                tu = tpool.tile([128, cs], dt.float32, tag="tu")
                nc.scalar.activation(tu[:], xg[:, :, 1], AF.Copy,
                                     bias=1.5, scale=0.5)
                tv = tpool.tile([128, cs], dt.float32, tag="t0")
                nc.scalar.activation(tv[:], xg[:, :, 1], AF.Copy,
                                     bias=96.0, scale=32.0)

                q0x4 = dpool.tile([128, cs], dt.bfloat16, tag="q0x4")
                nc.gpsimd.tensor_scalar(q0x4[:], t0[:], M4, -M4,
                                        op.add, op.add)
                q1x16 = dpool.tile([128, cs], dt.bfloat16, tag="q1x16")
                nc.gpsimd.tensor_scalar(q1x16[:], t1[:], M16, -M16,
                                        op.add, op.add)
                q2 = dpool.tile([128, cs], dt.bfloat16, tag="q2")
                nc.gpsimd.tensor_scalar(q2[:], t2[:], M1, -M1,
                                        op.add, op.add)
                u = dpool.tile([128, cs], dt.bfloat16, tag="u")
                nc.gpsimd.tensor_scalar(u[:], tu[:], M1, -M1,
                                        op.add, op.add)
                u64 = dpool.tile([128, cs], dt.bfloat16, tag="u64")
                nc.gpsimd.tensor_scalar(u64[:], tv[:], M64, -M64,
                                        op.add, op.add)

                nc.gpsimd.tensor_tensor(hi[:, c0:c1], q0x4[:], u[:], op.add)
                s1 = dpool.tile([128, cs], dt.bfloat16, tag="s1")
                nc.gpsimd.tensor_tensor(s1[:], q1x16[:], q2[:], op.add)
                nc.gpsimd.tensor_tensor(lo_dst[:, c0:c1], s1[:], u64[:],
                                        op.subtract)

            for w in range(NW):
                lo = hlpool.tile([128, WS], dt.bfloat16, tag="lo")
                ohh_tiles = []
                for h in range(2):
                    hwi = w * 2 + h
                    hi = hlpool.tile([128, HWS], dt.bfloat16, tag="hi")
                    lo_dst = lo[:, h * HWS:(h + 1) * HWS]
                    ohh = ohhpool.tile([128, 16, HWS], dt.bfloat16, tag="ohh")
                    if hwi == 0:
                        bounds = [0, 256, HWS]
                    else:
                        bounds = [0, HWS]
                    for c0, c1 in zip(bounds[:-1], bounds[1:]):
                        prep_chunk(hwi, hi, lo_dst, c0, c1)
                        for r in range(16):
                            nc.vector.tensor_scalar(ohh[:, r, c0:c1],
                                                    hi[:, c0:c1], HI_T[r],
                                                    None, op.is_equal)
                    ohh_tiles.append(ohh)

                ohl = ohlpool.tile([128, 32, WS], dt.bfloat16, tag="ohl")
                # First/last window: emit lo rows per half-window span so PE
                # can start (resp. finish) half a window earlier.
                lob = [0, HWS, WS] if w in (0, NW - 1) else [0, WS]
                for c0, c1 in zip(lob[:-1], lob[1:]):
                    for r in range(32):
                        nc.vector.tensor_scalar(ohl[:, r, c0:c1],
                                                lo[:, c0:c1], LO_T[r],
                                                None, op.is_equal)

                ps = ps1pool.tile([32, WIN * 16], dt.float32, tag="ps")
                for b in range(WIN):
                    ohh = ohh_tiles[b // HW]
                    sh = (b % HW) * PJ
                    sw = b * PJ
                    for j in range(PJ):
                        nc.tensor.matmul(ps[:, b * 16:(b + 1) * 16],
                                         ohl[:, :, sw + j],
                                         ohh[:, :, sh + j],
                                         start=(j == 0), stop=(j == PJ - 1))
                nc.scalar.copy(cnt[:, w * WIN:(w + 1) * WIN, :],
                               ps[:].rearrange("p (b h) -> p b h", h=16))

                if w == 0:
                    ps2 = ps2pool.tile([CLS, BPC], dt.float32)
                for half in range(2):
                    for r in range(16):
                        nc.tensor.matmul(
                            ps2[:, w * WIN:(w + 1) * WIN],
                            w2[:, half, r * CLS:(r + 1) * CLS],
                            cnt[:, w * WIN:(w + 1) * WIN, r],
                            start=(half == 0 and r == 0),
                            stop=(half == 1 and r == 15))

            out = cpool.tile([CLS, BPC], dt.float32)
            nc.vector.tensor_scalar(out[:], ps2[:], 1.0 / N, bias[:],
                                    op.mult, op.add)
            nc.sync.dma_start(y_d[:], out[:])

    nc.compile()
    return nc


def _aux_inputs(W, b):
    from ml_dtypes import bfloat16 as bf16
    # w2[lo_idx, half, r*CLS + c] = hi/lo bf16 split of W[c, lin(lo_idx, r)]
    lin = np.zeros((32, 16), np.int64)
    for r in range(16):
        a, uu = r // 2 + 1, r % 2
        for r2 in range(32):
            k, c2 = r2 // 8 + 1, r2 % 8 + 1
            lin[r2, r] = 64 * (a - 1) + 8 * (4 * uu + k - 1) + (c2 - 1)
    W2 = W[:, lin].astype(np.float32)                  # (CLS, 32, 16)
    W2 = np.ascontiguousarray(W2.transpose(1, 2, 0))   # (32, 16, CLS)
    W2h = W2.astype(bf16)
    W2l = (W2 - W2h.astype(np.float32)).astype(bf16)
    w2 = np.ascontiguousarray(
        np.stack([W2h, W2l], axis=1).reshape(32, 2, 16 * CLS))
    bias = np.asarray(b, dtype=np.float32).reshape(CLS, 1)
    return w2, bias


def kernel(x, W, b):
    from concourse.bass_utils import run_bass_kernel_spmd

    x = np.asarray(x, dtype=np.float32)
    W = np.asarray(W, dtype=np.float32)
    b = np.asarray(b, dtype=np.float32)

    if "nc" not in _CACHE:
        _CACHE["nc"] = _build()
    nc = _CACHE["nc"]

    w2, bias = _aux_inputs(W, b)
    # x (B, N, 3) -> per core [128p, NHW, HW, PJ, 3] -> (128, NHW, HWS*3)
    xs = x.reshape(NCORES, NHW, HW, 128, PJ, 3).transpose(0, 3, 1, 2, 4, 5)
    xs = np.ascontiguousarray(xs).reshape(NCORES, 128, NHW, HWS * 3)
    in_maps = [
        {"x": xs[i], "w2": w2, "bias": bias}
        for i in range(NCORES)
    ]
    res = run_bass_kernel_spmd(nc, in_maps, list(range(NCORES)))
    return np.concatenate(
        [np.asarray(res.results[i]["y"]).T for i in range(NCORES)],
        axis=0).astype(np.float32)
